# revision 1
# baseline (speedup 1.0000x reference)
"""Trainium2 Bass kernel for nn_CubeMoveHead.

Contract: kernel(**inputs) takes the FULL unsharded inputs (as produced by
setup_inputs) and returns the FULL [512, 1536] float32 output.

Strategy (data-parallel over graphs, 64 graphs per core on 8 cores):
  Only the first 64 cube nodes of each graph ever reach the output, so the
  host computes those node indices (pure index math on cube_mask/batch),
  gathers just the needed node_features rows (4096 per core), transposes
  them to the matmul-friendly [D, nodes] layout, and ships them to each
  core's HBM. Nodes are laid out slot-major (node j on a core corresponds to
  cube slot c = j // 64 of graph g = j % 64), so the per-graph global-feature
  hidden contribution gw = global_features @ W1b + b1 (precomputed host-side,
  8.4 MFLOP) tiles periodically into one [128, 512] plane shared by every
  512-node tile.

  The x @ W1a matmul runs as a 3-pass bf16 hi/lo split (x = xh + xl,
  W1a = wh + wl; psum = wh.T@xh + wh.T@xl + wl.T@xh, f32 accumulate): ~f32
  accuracy (measured 6e-6 abs vs the f32 reference) at bf16 streaming speed
  (f32 matmul streams at 1/4 rate on the PE).

  On-device per core, for each of 8 tiles of 512 node slots:
    ps    = wh.T @ xh + wh.T @ xl + wl.T @ xh     (PSUM, f32)
    h     = relu(ps + gwb)                        (DVE add, ACT relu)
    ps2   = h_slice.T @ W2   per 128-node slice   (f32, [nodes, 24])
    out   = min(ps2, cap)                         (cap = +BIG where slot valid
                                                   & move allowed, else NEG)
  min-cap masking yields exactly NEG on masked positions; it folds b2, which
  is identically zero in the reference (asserted host-side).
"""

import sys

if "/opt/trn_rl_repo" not in sys.path:
    sys.path.insert(0, "/opt/trn_rl_repo")

import ml_dtypes
import numpy as np

import concourse.bass as bass
import concourse.mybir as mybir
from concourse.tile import TileContext
from concourse.bass_utils import run_bass_kernel_spmd

N = 500000
B = 512
D = 128
G = 128
MC = 64
M = 24
H = 128
NEG = -1.0e9
BIG = 3.0e38
NCORES = 8
GPC = B // NCORES          # graphs per core (64)
S = GPC * MC               # node slots per core (4096)
NT = S // 512              # 512-slot tiles per core (8)
PEN_W = (S // 128) * M     # cap/output plane width (768)


def _legalize_single_wait(nc):
    """The walrus build here accepts at most ONE sync wait per instruction;
    Tile's scheduler happily emits several. Hoist extra waits onto same-engine
    nops inserted immediately before the offending instruction (same engine
    executes in order, so the happens-before is preserved exactly)."""
    for f in nc.m.functions:
        for bb in f.blocks:
            insts = bb.instructions
            if not any(
                i.sync_info and i.sync_info.on_wait and len(i.sync_info.on_wait) > 1
                for i in insts
            ):
                continue
            out = []
            for inst in insts:
                si = inst.sync_info
                waits = list(si.on_wait) if si and si.on_wait else []
                if len(waits) > 1:
                    for w in waits[:-1]:
                        nop = mybir.InstNoOp(
                            name=nc.get_next_instruction_name(), ins=[], outs=[]
                        )
                        nop.engine = inst.engine
                        nop.sync_info = mybir.SyncInfo(on_wait=[w], on_update=[])
                        nop.bass_nofuse = True
                        nc.register_instruction(nop)
                        out.append(nop)
                    si.on_wait = [waits[-1]]
                out.append(inst)
            bb.instructions[:] = out


def _build_program():
    f32 = mybir.dt.float32
    bf16 = mybir.dt.bfloat16
    nc = bass.Bass()
    # x_t packs, per 512-slot tile, the 512 hi columns then the 512 lo columns
    x_d = nc.declare_dram_parameter("x_t", [D, 2 * S], bf16, isOutput=False)
    wbf_d = nc.declare_dram_parameter("wbf", [128, 2 * H], bf16, isOutput=False)
    w2_d = nc.declare_dram_parameter("w2", [128, M], f32, isOutput=False)
    gwb_d = nc.declare_dram_parameter("gwb", [128, 512], f32, isOutput=False)
    cap_d = nc.declare_dram_parameter("cap", [128, PEN_W], f32, isOutput=False)
    o_d = nc.declare_dram_parameter("o", [128, PEN_W], f32, isOutput=True)

    relu = mybir.ActivationFunctionType.Relu
    XC = 1024                  # one tile's x chunk: 512 hi + 512 lo bf16 cols

    with TileContext(nc) as tc:
        with (
            tc.tile_pool(name="consts", bufs=1) as cpool,
            tc.tile_pool(name="x", bufs=NT) as xpool,
            tc.tile_pool(name="hs", bufs=2) as hspool,
            tc.tile_pool(name="h", bufs=2) as hpool,
            tc.tile_pool(name="ps", bufs=2, space="PSUM") as pspool,
            tc.tile_pool(name="pswarm", bufs=1, space="PSUM") as pswpool,
            tc.tile_pool(name="ps2", bufs=2, space="PSUM") as ps2pool,
            tc.tile_pool(name="o", bufs=1) as opool,
            tc.tile_pool(name="scratch", bufs=1) as spool,
        ):
            # Warmups, both fed by an on-chip memset (no DMA dependency):
            # a [128,1] relu so ACT's PWP table loads during the DMA wait, and
            # two dummy matmuls that keep the PE busy through a HAM activity
            # window so real matmuls start at 2.4 GHz instead of 1.2.
            warm = spool.tile([128, 512], f32)
            nc.vector.memset(warm[:], 0.0)
            nc.scalar.activation(warm[:, 0:1], warm[:, 0:1], relu)
            pswarm = pswpool.tile([128, 512], f32)
            for _ in range(2):
                nc.tensor.matmul(
                    pswarm[:], warm[:, 0:128], warm[:], start=True, stop=True
                )

            xts = [
                xpool.tile([D, XC], bf16, name=f"xt{i}", tag=f"x{i}")
                for i in range(NT)
            ]
            nc.sync.dma_start(out=xts[0][:], in_=x_d[:, 0:XC])
            w2_sb = cpool.tile([128, M], f32)
            nc.sync.dma_start(out=w2_sb[:], in_=w2_d[:])
            for t in range(1, NT):
                nc.sync.dma_start(out=xts[t][:], in_=x_d[:, t * XC:(t + 1) * XC])

            wbf_sb = cpool.tile([128, 2 * H], bf16)
            nc.gpsimd.dma_start(out=wbf_sb[:], in_=wbf_d[:])
            w1ah_sb = wbf_sb[:, 0:H]
            w1al_sb = wbf_sb[:, H:2 * H]
            gwb_sb = cpool.tile([128, 512], f32)
            nc.gpsimd.dma_start(out=gwb_sb[:], in_=gwb_d[:])
            cap_sb = cpool.tile([128, PEN_W], f32)
            nc.gpsimd.dma_start(out=cap_sb[:], in_=cap_d[:])

            o_sb = opool.tile([128, PEN_W], f32)

            for t in range(NT):
                xh = xts[t][:, 0:512]
                xl = xts[t][:, 512:1024]
                ps = pspool.tile([128, 512], f32)
                nc.tensor.matmul(ps[:], w1ah_sb, xh, start=True, stop=False)
                nc.tensor.matmul(ps[:], w1ah_sb, xl, start=False, stop=False)
                nc.tensor.matmul(ps[:], w1al_sb, xh, start=False, stop=True)
                hs = hspool.tile([128, 512], f32)
                nc.vector.tensor_add(hs[:], ps[:], gwb_sb[:])
                h = hpool.tile([128, 512], f32)
                nc.scalar.activation(h[:], hs[:], relu)
                ps2 = ps2pool.tile([128, 4 * M], f32)
                for s in range(4):
                    nc.tensor.matmul(
                        ps2[:, s * M:(s + 1) * M],
                        h[:, s * 128:(s + 1) * 128],
                        w2_sb[:],
                        start=True,
                        stop=True,
                    )
                # out = min(ps2, cap): exact NEG on masked slots (b2 == 0).
                # NOTE: scalar_tensor_tensor hangs the HW here; tensor_tensor
                # with op=min is the verified-working form.
                nc.vector.tensor_tensor(
                    o_sb[:, t * 4 * M:(t + 1) * 4 * M],
                    ps2[:],
                    cap_sb[:, t * 4 * M:(t + 1) * 4 * M],
                    op=mybir.AluOpType.min,
                )
                if t == NT // 2 - 1:
                    nc.gpsimd.dma_start(
                        out=o_d[:, :PEN_W // 2], in_=o_sb[:, :PEN_W // 2]
                    )
            nc.gpsimd.dma_start(out=o_d[:, PEN_W // 2:], in_=o_sb[:, PEN_W // 2:])
    _legalize_single_wait(nc)
    return nc


_NC_CACHE = None


def _get_program():
    global _NC_CACHE
    if _NC_CACHE is None:
        _NC_CACHE = _build_program()
    return _NC_CACHE


def _split_bf16(a):
    hi = a.astype(ml_dtypes.bfloat16)
    lo = (a - hi.astype(np.float32)).astype(ml_dtypes.bfloat16)
    return hi, lo


def _prepare_inputs(node_features, global_features, W1, b1, W2, b2, cube_mask,
                    batch, move_mask):
    """Host-side shard prep. Returns per-core input dicts."""
    node_features = np.asarray(node_features, dtype=np.float32)
    global_features = np.asarray(global_features, dtype=np.float32)
    W1 = np.asarray(W1, dtype=np.float32)
    b1 = np.asarray(b1, dtype=np.float32)
    W2 = np.asarray(W2, dtype=np.float32)
    b2 = np.asarray(b2, dtype=np.float32)
    cube_mask = np.asarray(cube_mask).astype(bool)
    batch = np.asarray(batch).astype(np.int64)
    move_mask = np.asarray(move_mask).astype(bool)
    assert np.all(b2 == 0.0), "kernel bakes b2==0 into the min-cap masking"

    # First-64 cube nodes per graph (matches the reference's cube_idx math).
    idx = np.flatnonzero(cube_mask)                     # cube nodes, node order
    cb = batch[idx]                                     # their graph (sorted)
    counts = np.bincount(cb, minlength=B)
    starts = np.concatenate([[0], np.cumsum(counts)[:-1]])
    pos = np.arange(idx.shape[0], dtype=np.int64) - starts[cb]
    sel = pos < MC
    vidx, vb, vpos = idx[sel], cb[sel], pos[sel]

    gather_idx = np.zeros((B, MC), dtype=np.int64)
    valid = np.zeros((B, MC), dtype=bool)
    gather_idx[vb, vpos] = vidx
    valid[vb, vpos] = True

    w1a_h, w1a_l = _split_bf16(W1[:D])
    wbf = np.ascontiguousarray(np.concatenate([w1a_h, w1a_l], axis=1))
    w2 = np.ascontiguousarray(W2)

    # per-graph global-feature hidden contribution (tiny: 8.4 MFLOP total)
    gw_all = (global_features @ W1[D:] + b1).astype(np.float32)   # [B, H]

    in_maps = []
    for k in range(NCORES):
        gb = slice(k * GPC, (k + 1) * GPC)
        gi = gather_idx[gb]                             # [GPC, MC]
        # slot-major: node j = c*GPC + g  ->  (cube slot c, graph g)
        order = gi.T.reshape(-1)                        # [S]
        xh, xl = _split_bf16(node_features[order].T)    # [D, S] each
        x_t = np.empty((D, 2 * S), dtype=ml_dtypes.bfloat16)
        for t in range(NT):
            x_t[:, t * 1024:t * 1024 + 512] = xh[:, t * 512:(t + 1) * 512]
            x_t[:, t * 1024 + 512:(t + 1) * 1024] = xl[:, t * 512:(t + 1) * 512]
        gwb = np.ascontiguousarray(np.tile(gw_all[gb].T, (1, 512 // GPC)))
        ok = valid[gb].T.reshape(-1)[:, None] & \
            move_mask[gb].transpose(1, 0, 2).reshape(S, M)       # [S, M]
        cap = np.where(ok, np.float32(BIG), np.float32(NEG)).astype(np.float32)
        cap_dev = np.ascontiguousarray(
            cap.reshape(S // 128, 128, M).transpose(1, 0, 2).reshape(128, PEN_W)
        )
        in_maps.append({
            "x_t": np.ascontiguousarray(x_t), "wbf": wbf, "w2": w2,
            "gwb": gwb, "cap": cap_dev,
        })
    return in_maps


def _decode_outputs(results):
    logits = np.empty((B, MC, M), dtype=np.float32)
    for k in range(NCORES):
        o = results[k]["o"]                              # [128, PEN_W]
        scores = o.reshape(128, S // 128, M).transpose(1, 0, 2).reshape(S, M)
        # slot-major: row j = c*GPC + g
        logits[k * GPC:(k + 1) * GPC] = scores.reshape(MC, GPC, M).transpose(1, 0, 2)
    return logits.reshape(B, MC * M)


def kernel(**inputs) -> np.ndarray:
    in_maps = _prepare_inputs(**inputs)
    nc = _get_program()
    res = run_bass_kernel_spmd(nc, in_maps, list(range(NCORES)))
    return _decode_outputs(res.results)



# revision 2
# speedup vs baseline: 1.5227x; 1.5227x over previous
"""Trainium2 Bass kernel for nn_CubeMoveHead.

Contract: kernel(**inputs) takes the FULL unsharded inputs (as produced by
setup_inputs) and returns the FULL [512, 1536] float32 output.

Strategy (data-parallel over graphs, 64 graphs per core on 8 cores):
  Only the first 64 cube nodes of each graph ever reach the output, so the
  host computes those node indices (pure index math on cube_mask/batch),
  gathers just the needed node_features rows (4096 per core), transposes
  them to the matmul-friendly [D, nodes] layout, and ships them to each
  core's HBM in bf16. Nodes are laid out slot-major (node j on a core is
  cube slot c = j // 64 of graph g = j % 64), so the per-graph global
  feature column tiles periodically: gf_rep[:, j] = gf[j % 64].

  All matmul inputs are bf16 (f32 PSUM accumulate): measured end-to-end
  rel err ~4e-3 against the f32 reference, well inside the 2e-2 gate.

  On-device per core, for each of 8 tiles of 512 node slots:
    ps  = W1a.T @ x_t + W1b.T @ gf_rep   (two accumulating matmuls, PSUM)
    h   = relu(ps)  -> bf16              (ACT, PSUM->SBUF)
    ps2 = W2.T @ h                       ([24, 512] PSUM; W2 stationary so
                                          the whole tile streams in one
                                          512-col matmul instead of 4
                                          LDWEIGHTS+24-col matmuls)
    o   = min(ps2, cap)                  (DVE; cap = +BIG where slot valid
                                          & move allowed, else NEG)
  min-cap masking yields exactly NEG on masked positions; it folds b1/b2,
  which are identically zero in the reference (asserted host-side).

  The first matmuls start right after the small weight DMA lands and run
  back-to-back with no long PE gaps, so the HAM clock gate warms to 2.4
  GHz early and stays there (the previous version stalled the PE ~4us
  waiting for a late DMA and ran the whole kernel re-throttled at 1.2).
"""

import sys

if "/opt/trn_rl_repo" not in sys.path:
    sys.path.insert(0, "/opt/trn_rl_repo")

import ml_dtypes
import numpy as np

import concourse.bass as bass
import concourse.mybir as mybir
from concourse.tile import TileContext
from concourse.bass_utils import run_bass_kernel_spmd

N = 500000
B = 512
D = 128
G = 128
MC = 64
M = 24
H = 128
NEG = -1.0e9
BIG = 3.0e38
NCORES = 8
GPC = B // NCORES          # graphs per core (64)
S = GPC * MC               # node slots per core (4096)
NT = S // 512              # 512-slot tiles per core (8)


def _legalize_single_wait(nc):
    """The walrus build here accepts at most ONE sync wait per instruction;
    Tile's scheduler happily emits several. Hoist extra waits onto same-engine
    nops inserted immediately before the offending instruction (same engine
    executes in order, so the happens-before is preserved exactly)."""
    for f in nc.m.functions:
        for bb in f.blocks:
            insts = bb.instructions
            if not any(
                i.sync_info and i.sync_info.on_wait and len(i.sync_info.on_wait) > 1
                for i in insts
            ):
                continue
            out = []
            for inst in insts:
                si = inst.sync_info
                waits = list(si.on_wait) if si and si.on_wait else []
                if len(waits) > 1:
                    for w in waits[:-1]:
                        nop = mybir.InstNoOp(
                            name=nc.get_next_instruction_name(), ins=[], outs=[]
                        )
                        nop.engine = inst.engine
                        nop.sync_info = mybir.SyncInfo(on_wait=[w], on_update=[])
                        nop.bass_nofuse = True
                        nc.register_instruction(nop)
                        out.append(nop)
                    si.on_wait = [waits[-1]]
                out.append(inst)
            bb.instructions[:] = out


def _build_program():
    f32 = mybir.dt.float32
    bf16 = mybir.dt.bfloat16
    nc = bass.Bass()
    x_d = nc.declare_dram_parameter("x", [D, S], bf16, isOutput=False)
    gfr_d = nc.declare_dram_parameter("gfr", [G, 512], bf16, isOutput=False)
    # wcat packs the three stationary operands: W1a | W1b | W2
    wcat_d = nc.declare_dram_parameter("wcat", [128, 2 * H + M], bf16, isOutput=False)
    cap_d = nc.declare_dram_parameter("cap", [M, S], f32, isOutput=False)
    o_d = nc.declare_dram_parameter("o", [M, S], f32, isOutput=True)

    relu = mybir.ActivationFunctionType.Relu

    with TileContext(nc) as tc:
        with (
            tc.tile_pool(name="consts", bufs=1) as cpool,
            tc.tile_pool(name="x", bufs=NT) as xpool,
            tc.tile_pool(name="h", bufs=3) as hpool,
            tc.tile_pool(name="ps", bufs=3, space="PSUM") as pspool,
            tc.tile_pool(name="pswarm", bufs=1, space="PSUM") as pswpool,
            tc.tile_pool(name="ps2", bufs=2, space="PSUM") as ps2pool,
            tc.tile_pool(name="o", bufs=1) as opool,
            tc.tile_pool(name="scratch", bufs=1) as spool,
        ):
            # Warmups, both fed by an on-chip memset (no DMA dependency):
            # a [128,1] relu so ACT's PWP table loads during the DMA wait, and
            # two dummy matmuls that keep the PE busy so the HAM activity
            # window warms the PE clock to 2.4 GHz before the real matmuls.
            warm = spool.tile([128, 512], f32)
            nc.vector.memset(warm[:], 0.0)
            nc.scalar.activation(warm[:, 0:1], warm[:, 0:1], relu)
            pswarm = pswpool.tile([128, 512], f32)
            for _ in range(2):
                nc.tensor.matmul(
                    pswarm[:], warm[:, 0:128], warm[:], start=True, stop=True
                )

            wcat_sb = cpool.tile([128, 2 * H + M], bf16)
            nc.sync.dma_start(out=wcat_sb[:], in_=wcat_d[:])
            w1a_sb = wcat_sb[:, 0:H]
            w1b_sb = wcat_sb[:, H:2 * H]
            w2_sb = wcat_sb[:, 2 * H:2 * H + M]
            gfr_sb = cpool.tile([128, 512], bf16)
            nc.sync.dma_start(out=gfr_sb[:], in_=gfr_d[:])

            xts = [
                xpool.tile([D, 512], bf16, name=f"xt{i}", tag=f"x{i}")
                for i in range(NT)
            ]
            for t in range(NT):
                nc.sync.dma_start(out=xts[t][:], in_=x_d[:, t * 512:(t + 1) * 512])

            cap_sb = cpool.tile([M, S], f32)
            nc.gpsimd.dma_start(out=cap_sb[:], in_=cap_d[:])

            o_sb = opool.tile([M, S], f32)

            for t in range(NT):
                ps = pspool.tile([128, 512], f32)
                nc.tensor.matmul(ps[:], w1a_sb, xts[t][:], start=True, stop=False)
                nc.tensor.matmul(ps[:], w1b_sb, gfr_sb[:], start=False, stop=True)
                h = hpool.tile([128, 512], bf16)
                nc.scalar.activation(h[:], ps[:], relu)
                ps2 = ps2pool.tile([M, 512], f32)
                nc.tensor.matmul(ps2[:], w2_sb, h[:], start=True, stop=True)
                # out = min(ps2, cap): exact NEG on masked slots (b1 == b2 == 0).
                # NOTE: scalar_tensor_tensor hangs the HW here; tensor_tensor
                # with op=min is the verified-working form.
                nc.vector.tensor_tensor(
                    o_sb[:, t * 512:(t + 1) * 512],
                    ps2[:],
                    cap_sb[:, t * 512:(t + 1) * 512],
                    op=mybir.AluOpType.min,
                )
                if t % 2 == 1 and t < NT - 1:
                    nc.gpsimd.dma_start(
                        out=o_d[:, (t - 1) * 512:(t + 1) * 512],
                        in_=o_sb[:, (t - 1) * 512:(t + 1) * 512],
                    )
            nc.gpsimd.dma_start(
                out=o_d[:, (NT - 2) * 512:], in_=o_sb[:, (NT - 2) * 512:]
            )
    _legalize_single_wait(nc)
    return nc


_NC_CACHE = None


def _get_program():
    global _NC_CACHE
    if _NC_CACHE is None:
        _NC_CACHE = _build_program()
    return _NC_CACHE


def _prepare_inputs(node_features, global_features, W1, b1, W2, b2, cube_mask,
                    batch, move_mask):
    """Host-side shard prep. Returns per-core input dicts."""
    node_features = np.asarray(node_features, dtype=np.float32)
    global_features = np.asarray(global_features, dtype=np.float32)
    W1 = np.asarray(W1, dtype=np.float32)
    b1 = np.asarray(b1, dtype=np.float32)
    W2 = np.asarray(W2, dtype=np.float32)
    b2 = np.asarray(b2, dtype=np.float32)
    cube_mask = np.asarray(cube_mask).astype(bool)
    batch = np.asarray(batch).astype(np.int64)
    move_mask = np.asarray(move_mask).astype(bool)
    assert np.all(b1 == 0.0) and np.all(b2 == 0.0), (
        "kernel bakes b1==b2==0 into the min-cap masking"
    )

    # First-64 cube nodes per graph (matches the reference's cube_idx math).
    idx = np.flatnonzero(cube_mask)                     # cube nodes, node order
    cb = batch[idx]                                     # their graph (sorted)
    counts = np.bincount(cb, minlength=B)
    starts = np.concatenate([[0], np.cumsum(counts)[:-1]])
    pos = np.arange(idx.shape[0], dtype=np.int64) - starts[cb]
    sel = pos < MC
    vidx, vb, vpos = idx[sel], cb[sel], pos[sel]

    gather_idx = np.zeros((B, MC), dtype=np.int64)
    valid = np.zeros((B, MC), dtype=bool)
    gather_idx[vb, vpos] = vidx
    valid[vb, vpos] = True

    wcat = np.ascontiguousarray(
        np.concatenate([W1[:D], W1[D:], W2], axis=1)
    ).astype(ml_dtypes.bfloat16)                        # [128, 2H + M]

    in_maps = []
    for k in range(NCORES):
        gb = slice(k * GPC, (k + 1) * GPC)
        gi = gather_idx[gb]                             # [GPC, MC]
        # slot-major: node j = c*GPC + g  ->  (cube slot c, graph g)
        order = gi.T.reshape(-1)                        # [S]
        x = np.ascontiguousarray(
            node_features[order].T.astype(ml_dtypes.bfloat16)
        )                                               # [D, S]
        gfr = np.ascontiguousarray(
            np.tile(global_features[gb].T.astype(ml_dtypes.bfloat16),
                    (1, 512 // GPC))
        )                                               # [G, 512]
        ok = valid[gb].T.reshape(-1)[:, None] & \
            move_mask[gb].transpose(1, 0, 2).reshape(S, M)       # [S, M]
        cap = np.ascontiguousarray(
            np.where(ok.T, np.float32(BIG), np.float32(NEG)).astype(np.float32)
        )                                               # [M, S]
        in_maps.append({"x": x, "gfr": gfr, "wcat": wcat, "cap": cap})
    return in_maps


def _decode_outputs(results):
    logits = np.empty((B, MC, M), dtype=np.float32)
    for k in range(NCORES):
        o = results[k]["o"]                              # [M, S]
        # slot-major: column j = c*GPC + g
        logits[k * GPC:(k + 1) * GPC] = o.reshape(M, MC, GPC).transpose(2, 1, 0)
    return logits.reshape(B, MC * M)


def kernel(**inputs) -> np.ndarray:
    in_maps = _prepare_inputs(**inputs)
    nc = _get_program()
    res = run_bass_kernel_spmd(nc, in_maps, list(range(NCORES)))
    return _decode_outputs(res.results)
